# revision 1
# baseline (speedup 1.0000x reference)
"""Gated linear attention (GLA) Bass kernel for Trainium2, 8 NeuronCores.

Sharding: one core per (batch, head) pair -- B=2 x H=4 = 8 cores.
Each core computes its head's full pipeline with a chunked-parallel form of
the gated recurrence (chunk = 128), entirely on-device:

  z   = x @ (Wgk1@Wgk2)          (fused on host)
  sp  = softplus(-(z + bgk2))    = -log_sigmoid(z + bgk2)
  G   = -cumsum_per_chunk(sp)/16 (inclusive)
  qt  = (x @ Wq)^T * exp(G)*scale ; kt = (x @ Wk)^T * exp(-G)
  per chunk c:  AT = kt^T-block' qt-block  (masked s<=t)
                O  = AT^T @ V + qt^T @ S ;  S = (S + k~^T V) * exp(G_last)
  o   = O * rsqrt(mean(O^2)+eps) * (g*sigmoid(g))   [g = x @ Wg]
  out = o @ ((gnorm*Wo_head) @ Whead)               (fused on host)

Host gathers: out[b] = sum_h core_out[b,h] + bhead.
"""
import sys, os
sys.path.insert(0, "/opt/trn_rl_repo")

import numpy as np

B, T, D = 2, 2048, 512
H = 4
dk, dv = 64, 128          # per-head key/value dims
C = 128                   # chunk length
GATE_NORM = 16.0
EPS = 1e-5
SCALE = dk ** -0.5

_CACHE = {}
BF16_CHUNK = False  # bf16 chunk matmuls: 141us vs 151us but 3.5e-3 err - rejected


def build(t=T):
    import concourse.bass as bass  # noqa: F401
    from concourse import bacc, mybir
    import concourse.tile as tile
    import concourse.hw_specs as hw_specs

    F32 = mybir.dt.float32
    F32R = mybir.dt.float32r
    BF16 = mybir.dt.bfloat16
    AF = mybir.ActivationFunctionType
    OP = mybir.AluOpType
    bf = BF16_CHUNK

    # All activation funcs used here (Exp, Ln, Square, Copy, Identity) live
    # together in the natural_log_exp_and_others table, but the table chooser
    # assigns each func to the first table containing it (Exp -> exp_and_others,
    # Ln -> natural_log), which thrashes ACT_TABLE_LOADs between every Ln/Exp
    # pair (measured 41 loads, 52us).  Steer the chooser by removing our funcs
    # from every other table in the cached table dict (indices are preserved,
    # so act_func_set_id stays aligned with the compiler's act_info.json).
    need = {AF.Exp, AF.Ln, AF.Square, AF.Copy, AF.Identity}
    keep = "natural_log_exp_and_others"
    tabs = hw_specs.get_activation_tables("gen3")
    if keep in tabs and need <= tabs[keep]:
        for name, s in tabs.items():
            if name != keep:
                s -= need

    nch = t // C              # chunks
    nts = t // 512            # 512-wide time slices
    assert t % 512 == 0

    nc = bacc.Bacc("TRN2", target_bir_lowering=False, debug=False)

    xt_d = nc.dram_tensor("xt", [128, 4, t], F32R, kind="ExternalInput")
    wqk_d = nc.dram_tensor("wqk", [128, 4, 2 * dk], F32R, kind="ExternalInput")
    wvg_d = nc.dram_tensor("wvg", [128, 4, 2 * dv], F32R, kind="ExternalInput")
    wgk_d = nc.dram_tensor("wgk12", [128, 4, dk], F32R, kind="ExternalInput")
    wf_d = nc.dram_tensor("wfused", [dv, 10], F32, kind="ExternalInput")
    nb_d = nc.dram_tensor("nbgk2", [dk, 1], F32, kind="ExternalInput")
    um_d = nc.dram_tensor("umask", [C, C], F32, kind="ExternalInput")
    id_d = nc.dram_tensor("ident", [128, 128], F32, kind="ExternalInput")
    out_d = nc.dram_tensor("out10", [t, 10], F32, kind="ExternalOutput")

    with tile.TileContext(nc) as tc:
        with (
            tc.tile_pool(name="wt", bufs=1) as wt,
            tc.tile_pool(name="big", bufs=1) as big,
            tc.tile_pool(name="sm", bufs=3) as sm,
            tc.tile_pool(name="ck", bufs=5) as ck,
            tc.tile_pool(name="pp", bufs=4, space="PSUM") as pp,
            tc.tile_pool(name="pc", bufs=4, space="PSUM") as pc,
        ):
            # ---- weights / consts (small, gpsimd queue) ----
            wqk_sb = wt.tile([128, 4, 2 * dk], F32R)
            wvg_sb = wt.tile([128, 4, 2 * dv], F32R)
            wgk_sb = wt.tile([128, 4, dk], F32R)
            wf_sb = wt.tile([dv, 10], F32)
            nb_sb = wt.tile([dk, 1], F32)
            um_sb = wt.tile([C, C], F32)
            id_sb = wt.tile([128, 128], F32)
            # matmul weights on the fast sync queue ahead of the x^T stream;
            # small consts on the gpsimd queue in parallel
            nc.sync.dma_start(wgk_sb[:], wgk_d[:])
            nc.sync.dma_start(wqk_sb[:], wqk_d[:])
            nc.sync.dma_start(wvg_sb[:], wvg_d[:])
            nc.gpsimd.dma_start(wf_sb[:], wf_d[:])
            nc.gpsimd.dma_start(nb_sb[:], nb_d[:])
            nc.gpsimd.dma_start(um_sb[:], um_d[:])
            nc.gpsimd.dma_start(id_sb[:], id_d[:])
            eps_sb = wt.tile([128, 1], F32)
            nc.vector.memset(eps_sb[:], EPS)
            lnsc_sb = wt.tile([dk, 1], F32)
            nc.vector.memset(lnsc_sb[:], float(np.log(SCALE)))
            if bf:
                idb_sb = wt.tile([128, 128], BF16)
                nc.vector.tensor_copy(idb_sb[:], id_sb[:])

            # ---- big SBUF tensors ----
            xT = big.tile([128, 4, t], F32R)      # x^T per 128-d-chunk
            qt = big.tile([dk, t], F32)           # q-tilde transposed
            kt = big.tile([dk, t], F32)           # k-tilde transposed
            if bf:
                qtb = big.tile([dk, t], BF16)
                ktb = big.tile([dk, t], BF16)
            sp = big.tile([dk, t], F32)
            spc = big.tile([dk, t], F32)
            dlast = big.tile([dk, nch], F32)
            vg = big.tile([128, nch, 2 * dv], F32)               # v | g
            sw = big.tile([128, nch, dv], F32)    # g*sigmoid(g)

            spc_v = spc[:].rearrange("p (c l) -> p c l", l=C)

            # scan reset mask: 0 at chunk starts, 1 elsewhere -> one scan per
            # 512-slice does 4 independent per-chunk cumsums
            mres = wt.tile([dk, 512], F32)
            nc.vector.memset(mres[:], 1.0)
            mres_v = mres[:].rearrange("p (c l) -> p c l", l=C)
            nc.vector.memset(mres_v[:, :, 0:1], 0.0)
            ones_sb = wt.tile([dk, 1], F32)
            nc.vector.memset(ones_sb[:], 1.0)

            # ---- x^T load (HWDGE; host supplies transposed x). First slice
            # split into 128-col pieces so the first matmuls start sooner.
            for i in range(4):
                nc.sync.dma_start(xT[:, :, i * C:(i + 1) * C],
                                  xt_d[:, :, i * C:(i + 1) * C])
            for j in range(1, nts):
                nc.sync.dma_start(xT[:, :, j * 512:(j + 1) * 512],
                                  xt_d[:, :, j * 512:(j + 1) * 512])

            def emit_proj(j):
                ts = slice(j * 512, (j + 1) * 512)
                # gate chain: z -> sp = ln(1+exp(-z-b)) -> masked-reset cumsum
                pg = pp.tile([dk, 512], F32, tag="P")
                for d4 in range(4):
                    nc.tensor.matmul(pg[:], wgk_sb[:, d4, :], xT[:, d4, ts],
                                     start=(d4 == 0), stop=(d4 == 3))
                eg = sm.tile([dk, 512], F32, tag="eg")
                nc.scalar.activation(out=eg[:], in_=pg[:], func=AF.Exp,
                                     scale=-1.0, bias=nb_sb[:])
                nc.scalar.activation(out=sp[:, ts], in_=eg[:], func=AF.Ln,
                                     bias=ones_sb[:])
                nc.vector.tensor_tensor_scan(
                    out=spc[:, ts], data0=mres[:], data1=sp[:, ts],
                    initial=0.0, op0=OP.mult, op1=OP.add)
                nc.scalar.activation(
                    out=dlast[:, 4 * j:4 * j + 4],
                    in_=spc_v[:, 4 * j:4 * j + 4, C - 1:C],
                    func=AF.Exp, scale=-1.0 / GATE_NORM)
                # decay factors, stacked [q-rows | k-rows] to match pqk psum
                ee = sm.tile([128, 512], F32, tag="ee")
                nc.scalar.activation(out=ee[0:dk, :], in_=spc[:, ts], func=AF.Exp,
                                     scale=-1.0 / GATE_NORM, bias=lnsc_sb[:])
                nc.scalar.activation(out=ee[dk:2 * dk, :], in_=spc[:, ts],
                                     func=AF.Exp, scale=1.0 / GATE_NORM)

                # q|k projection (fp32r), decay applied on psum eviction
                pqk = pp.tile([128, 512], F32, tag="P")
                for d4 in range(4):
                    nc.tensor.matmul(pqk[:], wqk_sb[:, d4, :], xT[:, d4, ts],
                                     start=(d4 == 0), stop=(d4 == 3))
                nc.vector.tensor_mul(out=qt[:, ts], in0=pqk[0:dk, :],
                                     in1=ee[0:dk, :])
                nc.vector.tensor_mul(out=kt[:, ts], in0=pqk[dk:2 * dk, :],
                                     in1=ee[dk:2 * dk, :])
                if bf:
                    nc.vector.tensor_copy(out=qtb[:, ts], in_=qt[:, ts])
                    nc.vector.tensor_copy(out=ktb[:, ts], in_=kt[:, ts])

                # v|g natural projections
                for i in range(4):
                    tt = 4 * j + i
                    pn = pp.tile([128, 2 * dv], F32, tag="P")
                    for d4 in range(4):
                        nc.tensor.matmul(pn[:],
                                         xT[:, d4, tt * C:(tt + 1) * C],
                                         wvg_sb[:, d4, :],
                                         start=(d4 == 0), stop=(d4 == 3))
                    nc.vector.tensor_copy(out=vg[:, tt, :], in_=pn[:])

                # swish(g) = g * sigmoid(g) = g / (1 + exp(-g))
                gsl = vg[:, 4 * j:4 * j + 4, dv:2 * dv]
                eg2 = sm.tile([128, 4, dv], F32, tag="eg2")
                nc.scalar.activation(out=eg2[:], in_=gsl, func=AF.Exp, scale=-1.0)
                nc.vector.tensor_scalar_add(out=eg2[:], in0=eg2[:], scalar1=1.0)
                sg2 = sm.tile([128, 4, dv], F32, tag="sg2")
                nc.vector.reciprocal_approx_fast(out=sg2[:], in_=eg2[:])
                nc.vector.tensor_mul(out=sw[:, 4 * j:4 * j + 4, :],
                                     in0=sg2[:], in1=gsl)

            for j in range(nts):
                emit_proj(j)

            # ---- chunked recurrence ----
            S_prev = ck.tile([dk, dv], F32, tag="S")
            nc.vector.memset(S_prev[:], 0.0)
            for c in range(nch):
                cs = slice(c * C, (c + 1) * C)
                v_c = vg[:, c, 0:dv]
                kt_c = (ktb if bf else kt)[:, cs]
                qt_c = (qtb if bf else qt)[:, cs]

                pat = pc.tile([C, C], F32, tag="C")
                nc.tensor.matmul(pat[:], kt_c, qt_c, start=True, stop=True)
                atm = ck.tile([C, C], BF16 if bf else F32, tag="atm")
                nc.vector.tensor_mul(out=atm[:], in0=pat[:], in1=um_sb[:])

                pkt = pc.tile([C, dk], BF16 if bf else F32, tag="C")
                nc.tensor.transpose(pkt[:], kt_c,
                                    (idb_sb if bf else id_sb)[0:dk, 0:dk])
                ktn = ck.tile([C, dk], BF16 if bf else F32, tag="ktn")
                nc.scalar.copy(ktn[:], pkt[:])

                po = pc.tile([C, dv], F32, tag="C")
                nc.tensor.matmul(po[:], atm[:], v_c, start=True, stop=False)
                nc.tensor.matmul(po[:], qt[:, cs], S_prev[:],
                                 start=False, stop=True)

                pds = pc.tile([dk, dv], F32, tag="C")
                nc.tensor.matmul(pds[:], ktn[:], v_c, start=True, stop=True)
                S_new = ck.tile([dk, dv], F32, tag="S")
                nc.vector.tensor_add(out=S_new[:], in0=S_prev[:], in1=pds[:])
                nc.vector.tensor_scalar_mul(out=S_new[:], in0=S_new[:],
                                            scalar1=dlast[:, c:c + 1])
                S_prev = S_new

                # rmsnorm + gate
                scr = ck.tile([C, dv], F32, tag="scr")
                ms = ck.tile([C, 1], F32, tag="ms")
                nc.scalar.activation(out=scr[:], in_=po[:], func=AF.Square,
                                     accum_out=ms[:])
                lnv = ck.tile([C, 1], F32, tag="lnv")
                nc.scalar.activation(out=lnv[:], in_=ms[:], func=AF.Ln,
                                     scale=1.0 / dv, bias=eps_sb[:])
                rstd = ck.tile([C, 1], F32, tag="rstd")
                nc.scalar.activation(out=rstd[:], in_=lnv[:], func=AF.Exp,
                                     scale=-0.5)
                on = ck.tile([C, dv], F32, tag="on")
                nc.scalar.mul(on[:], po[:], rstd[:])
                nc.vector.tensor_mul(out=on[:], in0=on[:], in1=sw[:, c, :])

                # transpose + fused output head
                pot = pc.tile([dv, C], F32, tag="C")
                nc.tensor.transpose(pot[:], on[:], id_sb[:])
                ots = ck.tile([dv, C], F32, tag="ots")
                nc.scalar.copy(ots[:], pot[:])
                p10 = pc.tile([C, 10], F32, tag="C")
                nc.tensor.matmul(p10[:], ots[:], wf_sb[:], start=True, stop=True)
                o10 = ck.tile([C, 10], F32, tag="o10")
                nc.vector.tensor_copy(o10[:], p10[:])
                nc.sync.dma_start(out_d[cs, :], o10[:])

    nc.compile()
    return nc


def _prep_inputs(inputs, t=T):
    """Per-core input dicts: core = 4*b + h."""
    ins = {k: np.ascontiguousarray(np.asarray(v, dtype=np.float32))
           for k, v in inputs.items()}
    x, Wq, Wk, Wv, Wg = ins["x"], ins["Wq"], ins["Wk"], ins["Wv"], ins["Wg"]
    Wgk12 = (ins["Wgk1"].astype(np.float64) @ ins["Wgk2"].astype(np.float64))
    bgk2, gnorm = ins["bgk2"], ins["gnorm_w"]
    Wo, Whead = ins["Wo"], ins["Whead"]
    nch = t // C

    um = (np.arange(C)[:, None] <= np.arange(C)[None, :]).astype(np.float32)
    ident = np.eye(128, dtype=np.float32)

    def chunk_w(w):  # [512, n] -> [128, 4, n]
        return np.ascontiguousarray(w.reshape(4, 128, -1).transpose(1, 0, 2))

    in_maps = []
    for core in range(8):
        b, h = divmod(core, 4)
        wf = ((gnorm[:, None].astype(np.float64)
               * Wo[h * dv:(h + 1) * dv, :].astype(np.float64))
              @ Whead.astype(np.float64)).astype(np.float32)
        in_maps.append({
            "xt": np.ascontiguousarray(
                x[b, :t].T.reshape(4, 128, t).transpose(1, 0, 2)),
            "wqk": chunk_w(np.concatenate(
                [Wq[:, h * dk:(h + 1) * dk], Wk[:, h * dk:(h + 1) * dk]], 1)),
            "wvg": chunk_w(np.concatenate(
                [Wv[:, h * dv:(h + 1) * dv], Wg[:, h * dv:(h + 1) * dv]], 1)),
            "wgk12": chunk_w(Wgk12[:, h * dk:(h + 1) * dk].astype(np.float32)),
            "wfused": np.ascontiguousarray(wf),
            "nbgk2": np.ascontiguousarray(-bgk2[h * dk:(h + 1) * dk, None]),
            "umask": um,
            "ident": ident,
        })
    return in_maps


def _gather(results, inputs, t=T):
    bhead = np.asarray(inputs["bhead"], dtype=np.float32)
    out = np.zeros((B, t, 10), np.float32)
    for core in range(8):
        b = core // 4
        out[b] += results[core]["out10"]
    out += bhead[None, None, :]
    return out


def run(inputs, trace=False, **kw):
    from concourse.bass_utils import run_bass_kernel_spmd
    if "nc" not in _CACHE:
        _CACHE["nc"] = build()
    nc = _CACHE["nc"]
    in_maps = _prep_inputs(inputs)
    res = run_bass_kernel_spmd(nc, in_maps, core_ids=list(range(8)),
                               trace=trace, **kw)
    return _gather(res.results, inputs), res


def kernel(**inputs) -> np.ndarray:
    out, _ = run(inputs, trace=False)
    return out



# revision 3
# speedup vs baseline: 1.9814x; 1.9814x over previous
"""Gated linear attention (GLA) Bass kernel for Trainium2, 8 NeuronCores.

Sharding: one core per (batch, head) pair -- B=2 x H=4 = 8 cores.
Each core computes its head's full pipeline with a chunked-parallel form of
the gated recurrence (chunk = 128), entirely on-device.

v2 design (vs v0 baseline at ~147us):
  - all matmuls in bf16 (1 cycle/row on PE vs 4 for fp32r at moving dim
    <256); fp32 PSUM accumulation throughout.  Host ships x^T and weights
    pre-cast to bf16 (halves the x DMA too).
  - O computed transposed ([dv, t]) so the output head matmul needs no
    per-chunk PE transpose; RMSNorm sum-of-squares via ones-matmul,
    rstd applied per-partition at the PSUM eviction of the head matmul.
  - the 16-step inter-chunk state recurrence S_c = S_{c-1}*e_c + D_c is
    computed by ONE tensor_tensor_scan over a [dk, (v, c)] layout
    (multiplier 0 at c=0 cuts the chain between v-rows), removing the
    serial DVE add/mul ping-pong from the schedule.
  - per-chunk matmuls batched 4-at-a-time into single PSUM banks
    (pat4/pkt4/pds4/po4T) so PSUM recycling no longer serializes chunks.

Math per core (b,h):
  z   = x @ (Wgk1@Wgk2)            (fused on host)
  sp  = softplus(-(z + bgk2)) ; spc = per-chunk inclusive cumsum
  qt  = (x@Wq)^T * exp(-spc/16)*scale ; kt = (x@Wk)^T * exp(+spc/16)
  per chunk c:  AT = kt_c^T-block qt_c-block (masked s<=t)
                D'_c = (ktn_c^T @ V_c) * e_c,  e_c = exp(-spc_last/16)
  S_c = S_{c-1}*e_c + D'_c         (one scan)
  O^T_c = V_c^T @ AT + S_{c-1}^T-as-lhsT @ qt_c   (PSUM accumulate)
  out10 = ((O^T * swish(g)^T)^T @ (gnorm*Wo_h@Whead)) * rstd
Host gathers: out[b] = sum_h core_out[b,h] + bhead.
"""
import sys, os
sys.path.insert(0, "/opt/trn_rl_repo")

import numpy as np

B, T, D = 2, 2048, 512
H = 4
dk, dv = 64, 128          # per-head key/value dims
C = 128                   # chunk length
GATE_NORM = 16.0
EPS = 1e-5
SCALE = dk ** -0.5

_CACHE = {}


def build(t=T):
    import concourse.bass as bass  # noqa: F401
    from concourse import bacc, mybir
    import concourse.tile as tile
    import concourse.hw_specs as hw_specs

    F32 = mybir.dt.float32
    BF16 = mybir.dt.bfloat16
    AF = mybir.ActivationFunctionType
    OP = mybir.AluOpType

    # Steer the activation-table chooser so every func we use (Exp, Ln,
    # Square, Copy, Identity) resolves to natural_log_exp_and_others --
    # otherwise Exp->exp_and_others vs Ln->natural_log thrashes
    # ACT_TABLE_LOADs (~1.3us each) between every pair.
    need = {AF.Exp, AF.Ln, AF.Square, AF.Copy, AF.Identity}
    keep = "natural_log_exp_and_others"
    tabs = hw_specs.get_activation_tables("gen3")
    if keep in tabs and need <= tabs[keep]:
        for name, s in tabs.items():
            if name != keep:
                s -= need

    nch = t // C              # chunks
    ngr = t // 512            # chunk groups of 4 / 512-wide time slices
    assert t % 512 == 0

    nc = bacc.Bacc("TRN2", target_bir_lowering=False, debug=False)

    xt_d = nc.dram_tensor("xt", [128, 4, t], BF16, kind="ExternalInput")
    wqk_d = nc.dram_tensor("wqk", [128, 4, 2 * dk], BF16, kind="ExternalInput")
    wv_d = nc.dram_tensor("wv", [128, 4, dv], BF16, kind="ExternalInput")
    wg_d = nc.dram_tensor("wg", [128, 4, dv], BF16, kind="ExternalInput")
    wgk_d = nc.dram_tensor("wgk12", [128, 4, dk], BF16, kind="ExternalInput")
    wf_d = nc.dram_tensor("wfused", [dv, 10], BF16, kind="ExternalInput")
    nb_d = nc.dram_tensor("nbgk2", [dk, 1], F32, kind="ExternalInput")
    um_d = nc.dram_tensor("umask", [C, C], F32, kind="ExternalInput")
    id_d = nc.dram_tensor("identb", [dk, dk], BF16, kind="ExternalInput")
    out_d = nc.dram_tensor("out10", [t, 10], F32, kind="ExternalOutput")

    with tile.TileContext(nc) as tc:
        with (
            tc.tile_pool(name="wt", bufs=1) as wt,
            tc.tile_pool(name="big", bufs=1) as big,
            tc.tile_pool(name="sm", bufs=4) as sm,
            tc.tile_pool(name="e2", bufs=3) as e2,
            tc.tile_pool(name="pp", bufs=3, space="PSUM") as pp,
            tc.tile_pool(name="pc", bufs=3, space="PSUM") as pc,
            tc.tile_pool(name="ps", bufs=2, space="PSUM") as ps,
        ):
            # ---- weights / consts ----
            wqk_sb = wt.tile([128, 4, 2 * dk], BF16)
            wv_sb = wt.tile([128, 4, dv], BF16)
            wg_sb = wt.tile([128, 4, dv], BF16)
            wgk_sb = wt.tile([128, 4, dk], BF16)
            wf_sb = wt.tile([dv, 10], BF16)
            nb_sb = wt.tile([dk, 1], F32)
            um_sb = wt.tile([C, C], F32)
            idb_sb = wt.tile([dk, dk], BF16)
            nc.sync.dma_start(wgk_sb[:], wgk_d[:])
            nc.sync.dma_start(wqk_sb[:], wqk_d[:])
            nc.sync.dma_start(wv_sb[:], wv_d[:])
            nc.sync.dma_start(wg_sb[:], wg_d[:])
            nc.gpsimd.dma_start(wf_sb[:], wf_d[:])
            nc.gpsimd.dma_start(nb_sb[:], nb_d[:])
            nc.gpsimd.dma_start(um_sb[:], um_d[:])
            nc.gpsimd.dma_start(idb_sb[:], id_d[:])
            eps_sb = wt.tile([128, 1], F32)
            nc.vector.memset(eps_sb[:], EPS)
            ones_sb = wt.tile([dk, 1], F32)
            nc.vector.memset(ones_sb[:], 1.0)
            lnsc_sb = wt.tile([dk, 1], F32)
            nc.vector.memset(lnsc_sb[:], float(np.log(SCALE)))
            onesb_sb = wt.tile([128, 1], BF16)
            nc.vector.memset(onesb_sb[:], 1.0)

            # scan reset mask for the intra-chunk gate cumsum: 0 at chunk
            # starts -> one scan per 512-slice does 4 independent cumsums
            mres = wt.tile([dk, 512], F32)
            nc.vector.memset(mres[:], 1.0)
            mres_v = mres[:].rearrange("p (c l) -> p c l", l=C)
            nc.vector.memset(mres_v[:, :, 0:1], 0.0)

            # ---- big SBUF tensors ----
            xT = big.tile([128, 4, t], BF16)      # x^T per 128-d-chunk
            qt = big.tile([dk, t], BF16)          # q-tilde transposed
            kt = big.tile([dk, t], BF16)          # k-tilde transposed
            sp = big.tile([dk, t], F32)
            spc = big.tile([dk, t], F32)
            dlast = big.tile([dk, nch], F32)
            vnat = big.tile([128, nch, dv], BF16)  # v natural per chunk
            gt = big.tile([dv, t], BF16)          # g^T
            sw = big.tile([dv, t], BF16)          # swish(g)^T
            atm = big.tile([C, nch, C], BF16)     # masked AT per chunk
            ktn = big.tile([C, nch, dk], BF16)    # k-tilde natural
            Dall = big.tile([dk, dv, nch], F32)   # D'_c, c fastest
            Escn = big.tile([dk, dv, nch], F32)   # scan multipliers
            Sall = big.tile([dk, dv, nch], F32)   # scanned states
            Sb = big.tile([dk, nch - 1, dv], BF16)  # S_c in bf16, c=0..14

            spc_v = spc[:].rearrange("p (c l) -> p c l", l=C)

            nc.vector.memset(Escn[:, :, 0:1], 0.0)

            # ---- x^T load (first slice split so matmuls start sooner) ----
            for i in range(4):
                nc.sync.dma_start(xT[:, :, i * C:(i + 1) * C],
                                  xt_d[:, :, i * C:(i + 1) * C])
            for j in range(1, ngr):
                nc.sync.dma_start(xT[:, :, j * 512:(j + 1) * 512],
                                  xt_d[:, :, j * 512:(j + 1) * 512])

            def emit_phase1(j):
                ts = slice(j * 512, (j + 1) * 512)
                # gate chain: z -> sp = ln(1+exp(-z-b)) -> masked-reset cumsum
                pg = pp.tile([dk, 512], F32, tag="P")
                for d4 in range(4):
                    nc.tensor.matmul(pg[:], wgk_sb[:, d4, :], xT[:, d4, ts],
                                     start=(d4 == 0), stop=(d4 == 3))
                eg = sm.tile([dk, 512], F32, tag="eg")
                nc.scalar.activation(out=eg[:], in_=pg[:], func=AF.Exp,
                                     scale=-1.0, bias=nb_sb[:])
                nc.scalar.activation(out=sp[:, ts], in_=eg[:], func=AF.Ln,
                                     bias=ones_sb[:])
                nc.vector.tensor_tensor_scan(
                    out=spc[:, ts], data0=mres[:], data1=sp[:, ts],
                    initial=0.0, op0=OP.mult, op1=OP.add)
                nc.scalar.activation(
                    out=dlast[:, 4 * j:4 * j + 4],
                    in_=spc_v[:, 4 * j:4 * j + 4, C - 1:C],
                    func=AF.Exp, scale=-1.0 / GATE_NORM)
                # decay factors, stacked [q-rows | k-rows] to match pqk psum
                ee = sm.tile([128, 512], F32, tag="ee")
                nc.scalar.activation(out=ee[0:dk, :], in_=spc[:, ts],
                                     func=AF.Exp, scale=-1.0 / GATE_NORM,
                                     bias=lnsc_sb[:])
                nc.scalar.activation(out=ee[dk:2 * dk, :], in_=spc[:, ts],
                                     func=AF.Exp, scale=1.0 / GATE_NORM)

                # q|k projection, decay applied on psum eviction -> bf16
                pqk = pp.tile([128, 512], F32, tag="P")
                for d4 in range(4):
                    nc.tensor.matmul(pqk[:], wqk_sb[:, d4, :], xT[:, d4, ts],
                                     start=(d4 == 0), stop=(d4 == 3))
                nc.vector.tensor_mul(out=qt[:, ts], in0=pqk[0:dk, :],
                                     in1=ee[0:dk, :])
                nc.vector.tensor_mul(out=kt[:, ts], in0=pqk[dk:2 * dk, :],
                                     in1=ee[dk:2 * dk, :])

                # g^T projection + swish
                pgt = pp.tile([dv, 512], F32, tag="P")
                for d4 in range(4):
                    nc.tensor.matmul(pgt[:], wg_sb[:, d4, :], xT[:, d4, ts],
                                     start=(d4 == 0), stop=(d4 == 3))
                nc.scalar.copy(gt[:, ts], pgt[:])
                s1 = sm.tile([dv, 512], F32, tag="s1")
                nc.scalar.activation(out=s1[:], in_=gt[:, ts], func=AF.Exp,
                                     scale=-1.0)
                nc.vector.tensor_scalar_add(out=s1[:], in0=s1[:], scalar1=1.0)
                nc.vector.reciprocal_approx_fast(out=s1[:], in_=s1[:])
                nc.vector.tensor_mul(out=sw[:, ts], in0=s1[:], in1=gt[:, ts])

                # v natural projection (per chunk)
                pv4 = pp.tile([128, 4, dv], F32, tag="P")
                for i in range(4):
                    tt = 4 * j + i
                    for d4 in range(4):
                        nc.tensor.matmul(pv4[:, i, :],
                                         xT[:, d4, tt * C:(tt + 1) * C],
                                         wv_sb[:, d4, :],
                                         start=(d4 == 0), stop=(d4 == 3))
                nc.scalar.copy(vnat[:, 4 * j:4 * j + 4, :], pv4[:])

                # intra-chunk attention AT = kt_c^T-block qt_c-block, masked
                pat4 = pc.tile([C, 4, C], F32, tag="C")
                for i in range(4):
                    cs = slice((4 * j + i) * C, (4 * j + i + 1) * C)
                    nc.tensor.matmul(pat4[:, i, :], kt[:, cs], qt[:, cs],
                                     start=True, stop=True)
                nc.vector.tensor_mul(
                    out=atm[:, 4 * j:4 * j + 4, :], in0=pat4[:],
                    in1=um_sb[:][:, None, :].broadcast_to([C, 4, C]))

                # k-tilde natural (PE transpose) -> ktn
                pkt4 = pc.tile([C, 4, dk], BF16, tag="C")
                for i in range(4):
                    cs = slice((4 * j + i) * C, (4 * j + i + 1) * C)
                    nc.tensor.transpose(pkt4[:, i, :], kt[:, cs], idb_sb[:])
                nc.scalar.copy(ktn[:, 4 * j:4 * j + 4, :], pkt4[:])

                # D_c = ktn_c^T @ V_c ; scaled by e_c on eviction
                pds4 = pc.tile([dk, 4, dv], F32, tag="C")
                for i in range(4):
                    tt = 4 * j + i
                    nc.tensor.matmul(pds4[:, i, :], ktn[:, tt, :],
                                     vnat[:, tt, :], start=True, stop=True)
                for i in range(4):
                    tt = 4 * j + i
                    nc.scalar.mul(Dall[:, :, tt], pds4[:, i, :],
                                  dlast[:, tt:tt + 1])

                # scan multipliers: e_c broadcast along v (c=0 stays 0)
                lo = 1 if j == 0 else 0
                nc.gpsimd.tensor_copy(
                    out=Escn[:, :, 4 * j + lo:4 * j + 4],
                    in_=dlast[:][:, None, 4 * j + lo:4 * j + 4]
                        .broadcast_to([dk, dv, 4 - lo]))

            for j in range(ngr):
                emit_phase1(j)

            # ---- inter-chunk state scan: S_c = S_{c-1} * e_c + D'_c ----
            nc.vector.tensor_tensor_scan(
                out=Sall[:].rearrange("p v c -> p (v c)"),
                data0=Escn[:].rearrange("p v c -> p (v c)"),
                data1=Dall[:].rearrange("p v c -> p (v c)"),
                initial=0.0, op0=OP.mult, op1=OP.add)
            for c in range(nch - 1):
                nc.scalar.copy(Sb[:, c, :], Sall[:, :, c])

            # ---- phase 2: O^T, rmsnorm, gate, head ----
            def emit_phase2(g):
                gs = slice(g * 512, (g + 1) * 512)
                po4 = pc.tile([dv, 4, C], F32, tag="C")
                for i in range(4):
                    cc = 4 * g + i
                    cs = slice(cc * C, (cc + 1) * C)
                    first = (cc == 0)
                    nc.tensor.matmul(po4[:, i, :], vnat[:, cc, :],
                                     atm[:, cc, :], start=True, stop=first)
                    if not first:
                        nc.tensor.matmul(po4[:, i, :], Sb[:, cc - 1, :],
                                         qt[:, cs], start=False, stop=True)
                sq4 = e2.tile([dv, 512], BF16, tag="sq4")
                nc.scalar.activation(out=sq4[:], in_=po4[:], func=AF.Square)
                ot4 = e2.tile([dv, 512], BF16, tag="ot4")
                nc.vector.tensor_mul(out=ot4[:], in0=po4[:], in1=sw[:, gs])

                pms = ps.tile([C, 4], F32, tag="S")
                for i in range(4):
                    nc.tensor.matmul(pms[:, i:i + 1],
                                     sq4[:, i * C:(i + 1) * C],
                                     onesb_sb[:], start=True, stop=True)
                lnv = e2.tile([C, 4], F32, tag="lnv")
                nc.scalar.activation(out=lnv[:], in_=pms[:], func=AF.Ln,
                                     scale=1.0 / dv, bias=eps_sb[:])
                rstd = e2.tile([C, 4], F32, tag="rstd")
                nc.scalar.activation(out=rstd[:], in_=lnv[:], func=AF.Exp,
                                     scale=-0.5)

                p10 = ps.tile([C, 4, 10], F32, tag="S")
                for i in range(4):
                    nc.tensor.matmul(p10[:, i, :], ot4[:, i * C:(i + 1) * C],
                                     wf_sb[:], start=True, stop=True)
                o10 = e2.tile([C, 4, 10], F32, tag="o10")
                for i in range(4):
                    nc.scalar.mul(o10[:, i, :], p10[:, i, :],
                                  rstd[:, i:i + 1])
                nc.sync.dma_start(
                    out_d[gs, :].rearrange("(c p) j -> p c j", c=4), o10[:])

            for g in range(ngr):
                emit_phase2(g)

    nc.compile()
    return nc


def _prep_inputs(inputs, t=T):
    """Per-core input dicts: core = 4*b + h."""
    from ml_dtypes import bfloat16
    ins = {k: np.ascontiguousarray(np.asarray(v, dtype=np.float32))
           for k, v in inputs.items()}
    x, Wq, Wk, Wv, Wg = ins["x"], ins["Wq"], ins["Wk"], ins["Wv"], ins["Wg"]
    Wgk12 = (ins["Wgk1"].astype(np.float64) @ ins["Wgk2"].astype(np.float64))
    bgk2, gnorm = ins["bgk2"], ins["gnorm_w"]
    Wo, Whead = ins["Wo"], ins["Whead"]

    um = (np.arange(C)[:, None] <= np.arange(C)[None, :]).astype(np.float32)
    identb = np.eye(dk, dtype=bfloat16)

    def chunk_w(w):  # [512, n] -> [128, 4, n] bf16
        return np.ascontiguousarray(
            w.reshape(4, 128, -1).transpose(1, 0, 2)).astype(bfloat16)

    in_maps = []
    for core in range(8):
        b, h = divmod(core, 4)
        wf = ((gnorm[:, None].astype(np.float64)
               * Wo[h * dv:(h + 1) * dv, :].astype(np.float64))
              @ Whead.astype(np.float64)).astype(np.float32)
        in_maps.append({
            "xt": np.ascontiguousarray(
                x[b, :t].T.reshape(4, 128, t).transpose(1, 0, 2)
            ).astype(bfloat16),
            "wqk": chunk_w(np.concatenate(
                [Wq[:, h * dk:(h + 1) * dk], Wk[:, h * dk:(h + 1) * dk]], 1)),
            "wv": chunk_w(Wv[:, h * dv:(h + 1) * dv]),
            "wg": chunk_w(Wg[:, h * dv:(h + 1) * dv]),
            "wgk12": chunk_w(Wgk12[:, h * dk:(h + 1) * dk].astype(np.float32)),
            "wfused": np.ascontiguousarray(wf).astype(bfloat16),
            "nbgk2": np.ascontiguousarray(-bgk2[h * dk:(h + 1) * dk, None]),
            "umask": um,
            "identb": identb,
        })
    return in_maps


def _gather(results, inputs, t=T):
    bhead = np.asarray(inputs["bhead"], dtype=np.float32)
    out = np.zeros((B, t, 10), np.float32)
    for core in range(8):
        b = core // 4
        out[b] += results[core]["out10"]
    out += bhead[None, None, :]
    return out


def run(inputs, trace=False, **kw):
    from concourse.bass_utils import run_bass_kernel_spmd
    if "nc" not in _CACHE:
        _CACHE["nc"] = build()
    nc = _CACHE["nc"]
    in_maps = _prep_inputs(inputs)
    res = run_bass_kernel_spmd(nc, in_maps, core_ids=list(range(8)),
                               trace=trace, **kw)
    return _gather(res.results, inputs), res


def kernel(**inputs) -> np.ndarray:
    out, _ = run(inputs, trace=False)
    return out
